# revision 25
# baseline (speedup 1.0000x reference)
"""MoE experts kernel (grouped GEMM + SwiGLU) on 8 Trainium2 NeuronCores.

Problem: N=4096 tokens sorted by expert, E=8 experts, H=1024, I=2048, bf16.
  up    = gmm(hiddens, w13)            # [N, 2I]
  gated = silu(up[:, :I]) * up[:, I:]  # [N, I]
  down  = gmm(gated, w2)               # [N, H]

Sharding: expert parallelism. Core e owns expert e's weights and its
contiguous block of tokens (batch_sizes[e] each, 512 in the target regime).
No collectives needed; tokens are scattered/gathered on the host.

Per-core dataflow (all matmuls via TensorE, lhsT = stationary [K,128]):
  xT  [H, T]   <- DMA-transpose of x [T, H]           (one-time, 1MB)
  upT = w13.T-free: matmul(lhsT=w13[:,mchunk], rhs=xT)   -> PSUM [128, T]
  gatedT[c] = silu(upT_gate) * upT_up  (ACT Silu + DVE mul) -> SBUF bf16
  down = matmul(lhsT=gatedT[:,kc,mc*128:], rhs=w2[kc])  -> PSUM [128, 512]
Both weight matrices are consumed in their native K-major DRAM layout.
"""

import sys

if "/opt/trn_rl_repo" not in sys.path:
    sys.path.insert(0, "/opt/trn_rl_repo")

import numpy as np
import ml_dtypes

E = 8
H = 1024
I = 2048
N = 4096
T = N // E          # tokens per expert / core
P = 128
XA = T + P          # x input is augmented with 128 identity rows (host-built)
KH = H // P         # 8  k-subtiles for mm1
NI = I // P         # 16 k-subtiles for mm2 / gated chunks
FD = 512            # matmul moving free dim (1 PSUM bank of f32)
SW = 512            # w13 m-super-chunk width
BF16 = ml_dtypes.bfloat16

_NC_CACHE = {}


def _build_nc(act="silu", xpose="pe"):
    import concourse.bass as bass
    import concourse.tile as tile
    from concourse import mybir
    from concourse.vector_clock import ScopedClock, VectorClock

    class SplitDrainTileContext(tile.TileContext):
        """The walrus build in this environment rejects any instruction
        carrying more than ONE embedded sync wait ("Too many sync wait
        commands"). Tile's stock tail emits a single drain waiting on every
        active proc sem; split it into a chain of drains with one wait each.
        """

        def _drain_and_barrier(self, tick_clock, wait_clock):
            nc = self.nc
            gclock = tick_clock.global_clock
            n = len(gclock)
            for p in range(n):
                if gclock[p] <= 0:
                    continue
                masked = VectorClock([gclock[q] if q == p else 0
                                      for q in range(n)])
                d = nc.sync.drain()
                wait_clock.add_sem_waits(d.ins, ScopedClock({None: masked}))
            nc.all_engine_barrier()
            assert self.sems is not None
            popped = nc._tile_sem_poison_stack.pop()
            assert popped is self._sem_poison
            nc.clear_and_free_semaphores(list(self.sems.allocated().values()))
            nc.all_engine_barrier()

    nc = bass.Bass()
    bf = mybir.dt.bfloat16
    f32 = mybir.dt.float32

    # x rows [0, T) are tokens; rows [T, T+P) carry a host-built identity
    # block in columns [0, P) (used by the PE transpose — building it
    # on-device needs gpsimd, whose Pool sem pushes the tail drain past
    # the 10-sync-wait CTRL_NO limit of this compiler).
    x = nc.declare_dram_parameter("x", [XA, H], bf, isOutput=False)
    w13 = nc.declare_dram_parameter("w13", [H, 2 * I], bf, isOutput=False)
    w2 = nc.declare_dram_parameter("w2", [I, H], bf, isOutput=False)
    out = nc.declare_dram_parameter("out", [T, H], bf, isOutput=True)

    with SplitDrainTileContext(nc) as tc:
        with (
            tc.tile_pool(name="persist", bufs=1) as persist,
            tc.tile_pool(name="w13p", bufs=2) as w13p,
            tc.tile_pool(name="work", bufs=3) as work,
            # sg gets unique slots: avoids WAR sem-waits piling onto
            # instructions whose ISA structs have few sync-wait slots.
            tc.tile_pool(name="sgp", bufs=16) as sgp,
            tc.tile_pool(name="otp", bufs=1) as otp,
            tc.tile_pool(name="tch", bufs=16) as tch,
            tc.tile_pool(name="ps1", bufs=2, space="PSUM") as ps1,
            tc.tile_pool(name="ps2", bufs=2, space="PSUM") as ps2,
        ):
            # x [T, H] -> xT [P, KH, T] : xT[p, o, t] = x[t, o*P + p]
            xT = persist.tile([P, KH, T], bf)
            if xpose == "xbar":
                nc.sync.dma_start_transpose(xT[:], x[:T, :])
            else:
                # PE-based transpose: load x natively then 32x 128x128
                # tensor.transpose. Avoids the DMA XBAR path entirely.
                xrows = persist.tile([P, XA // P, H], bf)
                nc.sync.dma_start(
                    xrows[:], x.rearrange("(a p) h -> p a h", p=P)
                )
                ident = xrows[:, XA // P - 1, 0:P]
                with tc.tile_pool(name="pst", bufs=2, space="PSUM") as pst:
                    for o in range(KH):
                        for a in range(T // P):
                            pt = pst.tile([P, P], bf, tag="pt")
                            nc.tensor.transpose(
                                pt[:], xrows[:, a, o * P:(o + 1) * P], ident
                            )
                            nc.vector.tensor_copy(
                                xT[:, o, a * P:(a + 1) * P], pt[:]
                            )

            gatedT = persist.tile([P, NI, T], bf)
            w2s = persist.tile([P, NI, H], bf)
            nc.sync.dma_start(w2s[:], w2.rearrange("(o p) h -> p o h", p=P))

            # ---- mm1 + SwiGLU, paired gate/up super-chunks ----
            # 4 wide w13 loads (2 gate supers + 2 up supers of SWW cols):
            # keeps total HWDGE DMA count at 7 so (a) no queue is reused and
            # (b) the tail drain stays within the CTRL_NO sync-wait budget.
            SWW = 1024
            for s in range(I // SWW):  # 2
                w13g = w13p.tile([P, KH, SWW], bf, tag="w13g")
                nc.sync.dma_start(
                    w13g[:],
                    w13[:, s * SWW:(s + 1) * SWW].rearrange(
                        "(o p) m -> p o m", p=P
                    ),
                )
                w13u = w13p.tile([P, KH, SWW], bf, tag="w13u")
                nc.sync.dma_start(
                    w13u[:],
                    w13[:, I + s * SWW: I + (s + 1) * SWW].rearrange(
                        "(o p) m -> p o m", p=P
                    ),
                )
                for j in range(SWW // P):  # 8
                    pg = ps1.tile([P, T], f32, tag="pg")
                    pu = ps1.tile([P, T], f32, tag="pu")
                    for k in range(KH):
                        nc.tensor.matmul(
                            pg[:], w13g[:, k, j * P:(j + 1) * P], xT[:, k, :],
                            start=(k == 0), stop=(k == KH - 1),
                        )
                    for k in range(KH):
                        nc.tensor.matmul(
                            pu[:], w13u[:, k, j * P:(j + 1) * P], xT[:, k, :],
                            start=(k == 0), stop=(k == KH - 1),
                        )
                    sg = sgp.tile([P, T], bf, tag="sg")
                    # act="sigmoid" is a CoreSim-only variant (the sim has
                    # no Silu); identical dataflow, different curve.
                    fn = (mybir.ActivationFunctionType.Silu if act == "silu"
                          else mybir.ActivationFunctionType.Sigmoid)
                    nc.scalar.activation(sg[:], pg[:], fn)
                    # DVE compute instructions hold at most ONE sync wait,
                    # but the mul depends on both PE (pu) and ACT (sg). A
                    # tiny DVE copy of sg's first column absorbs the ACT
                    # wait first, so Tile's vector clock lets the mul carry
                    # only the PE wait.
                    touch = tch.tile([P, 1], bf, tag="touch")
                    nc.vector.tensor_copy(touch[:], sg[:, 0:1])
                    c = s * (SWW // P) + j
                    nc.vector.scalar_tensor_tensor(
                        gatedT[:, c, :], pu[:], 1.0, sg[:],
                        mybir.AluOpType.mult, mybir.AluOpType.mult,
                    )

            # ---- mm2: down[mc*P:, :] = gatedT.T @ w2 ----
            # All four token-chunks stage into one SBUF tile and leave in a
            # single ring-path DMA: a per-chunk store would lower to the
            # direct-2D DMA struct, which can't hold its two sem waits.
            obuf = otp.tile([P, T // P, H], bf, tag="obuf")
            for mc in range(T // P):  # 4
                for nh in range(H // FD):  # 2
                    pd = ps2.tile([P, FD], f32, tag="pd")
                    for kc in range(NI):
                        nc.tensor.matmul(
                            pd[:],
                            gatedT[:, kc, mc * P:(mc + 1) * P],
                            w2s[:, kc, nh * FD:(nh + 1) * FD],
                            start=(kc == 0), stop=(kc == NI - 1),
                        )
                    nc.vector.tensor_copy(
                        obuf[:, mc, nh * FD:(nh + 1) * FD], pd[:]
                    )
            nc.sync.dma_start(
                out.rearrange("(a p) h -> p a h", p=P), obuf[:]
            )

    return nc


def _get_nc():
    if "nc" not in _NC_CACHE:
        _NC_CACHE["nc"] = _build_nc()
    return _NC_CACHE["nc"]


def kernel(bs, hiddens, w13_weight, w2_weight, batch_sizes, **_ignored):
    from concourse.bass_utils import run_bass_kernel_spmd

    hiddens = np.asarray(hiddens)
    w13_weight = np.asarray(w13_weight)
    w2_weight = np.asarray(w2_weight)
    batch_sizes = np.asarray(batch_sizes).astype(np.int64)

    in_dtype = hiddens.dtype
    x = np.ascontiguousarray(hiddens.astype(BF16))
    w13 = np.ascontiguousarray(w13_weight.astype(BF16))
    w2 = np.ascontiguousarray(w2_weight.astype(BF16))

    assert batch_sizes.shape == (E,) and int(batch_sizes.sum()) == N, (
        "kernel compiled for 8 experts x 4096 tokens"
    )

    # Host-side scatter: expert e owns its contiguous token block. The
    # compiled kernel takes exactly T=512 tokens; pad/truncate-free fast
    # path requires the uniform routing produced by setup_inputs().
    offsets = np.concatenate([[0], np.cumsum(batch_sizes)])
    uniform = bool((batch_sizes == T).all())

    ident_rows = np.zeros((P, H), dtype=BF16)
    ident_rows[:P, :P] = np.eye(P, dtype=np.float32).astype(BF16)

    in_maps = []
    for e in range(E):
        xe = np.zeros((XA, H), dtype=BF16)
        if uniform:
            xe[:T] = x[e * T:(e + 1) * T]
        else:
            blk = x[offsets[e]:offsets[e + 1]]
            assert blk.shape[0] <= T, "per-expert batch exceeds compiled T"
            xe[: blk.shape[0]] = blk
        xe[T:] = ident_rows
        in_maps.append({"x": xe, "w13": w13[e], "w2": w2[e]})

    nc = _get_nc()
    results = run_bass_kernel_spmd(nc, in_maps, list(range(E))).results

    out_full = np.empty((N, H), dtype=BF16)
    for e in range(E):
        oe = np.asarray(results[e]["out"])
        if uniform:
            out_full[e * T:(e + 1) * T] = oe
        else:
            nb = int(batch_sizes[e])
            out_full[offsets[e]:offsets[e + 1]] = oe[:nb]

    return out_full.astype(in_dtype)


# revision 26
# speedup vs baseline: 1.1151x; 1.1151x over previous
"""MoE experts kernel (grouped GEMM + SwiGLU) on 8 Trainium2 NeuronCores.

Problem: N=4096 tokens sorted by expert, E=8 experts, H=1024, I=2048, bf16.
  up    = gmm(hiddens, w13)            # [N, 2I]
  gated = silu(up[:, :I]) * up[:, I:]  # [N, I]
  down  = gmm(gated, w2)               # [N, H]

Sharding: expert parallelism. Core e owns expert e's weights and its
contiguous block of tokens (batch_sizes[e] each; 512 in the target
regime). No collectives; tokens are scattered/gathered on the host.

Per-core dataflow (lhsT = stationary operand of nc.tensor.matmul):
  xT   [H, T] <- PE transpose of x (identity shipped from host inside x)
  upT  = matmul(lhsT=w13[:, chunk], rhs=xT)  -> PSUM [128, T]   (k = H)
  gatedT[c] = silu(upT_gate) * upT_up        -> SBUF bf16 chunks
  down = matmul(lhsT=gatedT[c], rhs=w2[c])   -> PSUM [128, 512] (k = I)
Both weights are consumed in native K-major DRAM layout; only x needs a
transpose, done on the PE against a host-supplied identity block.

Environment constraint that shaped everything here: this walrus build
rejects ANY instruction carrying more than one embedded sync wait. Hence
- all loads ride the single SWDGE lane (strict FIFO = priority order,
  zero waits on load DMAs),
- dummy PE "observer" transposes absorb DMA-progress waits so real
  matmuls only ever carry one wait,
- a custom TileContext splits the tail drain into one-wait chains.
"""

import sys

if "/opt/trn_rl_repo" not in sys.path:
    sys.path.insert(0, "/opt/trn_rl_repo")

import numpy as np
import ml_dtypes

E = 8
H = 1024
I = 2048
N = 4096
T = N // E          # tokens per expert / core
P = 128
XA = T + P          # x input is augmented with 128 identity rows
KH = H // P         # 8  k-subtiles for mm1
NI = I // P         # 16 k-subtiles for mm2 / gated chunks
FD = 512            # matmul moving free dim (1 PSUM bank of f32)
# w13 column-slab widths per half: small first so the first pairs start
# early while the SWDGE FIFO streams the rest.
SLABS = (128, 384, 512, 1024)
BF16 = ml_dtypes.bfloat16

_NC_CACHE = {}


def _slab_of(c):
    """Map gated-chunk index c (0..15) -> (slab_idx, col offset in slab)."""
    base = 0
    for si, w in enumerate(SLABS):
        n = w // P
        if c < n:
            return si, c * P
        c -= n
        base += w
    raise IndexError(c)


def _build_nc(act="silu"):
    import concourse.bass as bass
    import concourse.tile as tile
    from concourse import mybir
    from concourse.vector_clock import ScopedClock, VectorClock

    class SplitDrainTileContext(tile.TileContext):
        """Tail drain emitted as a chain of single-wait drains (the
        compiler rejects instructions with >1 embedded sync wait)."""

        def _drain_and_barrier(self, tick_clock, wait_clock):
            nc = self.nc
            gclock = tick_clock.global_clock
            n = len(gclock)
            for p in range(n):
                if gclock[p] <= 0:
                    continue
                masked = VectorClock([gclock[q] if q == p else 0
                                      for q in range(n)])
                d = nc.sync.drain()
                wait_clock.add_sem_waits(d.ins, ScopedClock({None: masked}))
            nc.all_engine_barrier()
            assert self.sems is not None
            popped = nc._tile_sem_poison_stack.pop()
            assert popped is self._sem_poison
            nc.clear_and_free_semaphores(list(self.sems.allocated().values()))
            nc.all_engine_barrier()

    nc = bass.Bass()
    bf = mybir.dt.bfloat16
    f32 = mybir.dt.float32

    x = nc.declare_dram_parameter("x", [XA, H], bf, isOutput=False)
    w13 = nc.declare_dram_parameter("w13", [H, 2 * I], bf, isOutput=False)
    w2 = nc.declare_dram_parameter("w2", [I, H], bf, isOutput=False)
    out = nc.declare_dram_parameter("out", [T, H], bf, isOutput=True)

    fn = (mybir.ActivationFunctionType.Silu if act == "silu"
          else mybir.ActivationFunctionType.Sigmoid)

    with SplitDrainTileContext(nc) as tc:
        with (
            tc.tile_pool(name="persist", bufs=1) as persist,
            tc.tile_pool(name="sgp", bufs=16) as sgp,
            tc.tile_pool(name="gtp", bufs=16) as gtp,
            tc.tile_pool(name="tch", bufs=16) as tch,
            tc.tile_pool(name="otp", bufs=1) as otp,
            tc.tile_pool(name="pst", bufs=2, space="PSUM") as pst,
            tc.tile_pool(name="ps1", bufs=2, space="PSUM") as ps1,
            tc.tile_pool(name="ps2", bufs=2, space="PSUM") as ps2,
        ):
            # ---- SWDGE load stream (one lane, FIFO = priority order) ----
            xrows = persist.tile([P, XA // P, H], bf)
            nc.gpsimd.dma_start(
                xrows[:], x.rearrange("(a p) h -> p a h", p=P)
            )
            ident = xrows[:, XA // P - 1, 0:P]

            w13g_slabs = []
            w13u_slabs = []
            off = 0
            for si, wdt in enumerate(SLABS):
                g = persist.tile([P, KH, wdt], bf, tag=f"w13g{si}")
                nc.gpsimd.dma_start(
                    g[:],
                    w13[:, off:off + wdt].rearrange("(o p) m -> p o m", p=P),
                )
                u = persist.tile([P, KH, wdt], bf, tag=f"w13u{si}")
                nc.gpsimd.dma_start(
                    u[:],
                    w13[:, I + off:I + off + wdt].rearrange(
                        "(o p) m -> p o m", p=P
                    ),
                )
                w13g_slabs.append(g)
                w13u_slabs.append(u)
                off += wdt

            w2s = persist.tile([P, NI, H], bf)
            nc.gpsimd.dma_start(w2s[:], w2.rearrange("(o p) h -> p o h", p=P))

            # ---- x transpose on PE: 32x 128x128 against host identity ----
            xT = persist.tile([P, KH, T], bf)
            for o in range(KH):
                for a in range(T // P):
                    pt = pst.tile([P, P], bf, tag="pt")
                    nc.tensor.transpose(
                        pt[:], xrows[:, a, o * P:(o + 1) * P], ident
                    )
                    nc.vector.tensor_copy(xT[:, o, a * P:(a + 1) * P], pt[:])

            # ---- mm1 + SwiGLU over 16 gate/up column-chunk pairs ----
            gts = []
            seen_slab = -1
            for c in range(NI):
                si, co = _slab_of(c)
                if si != seen_slab:
                    # Observer transposes: absorb the SWDGE-progress wait
                    # for this slab (g and u) on the PE so the real
                    # matmuls below carry only their single WAR wait.
                    for src in (w13g_slabs[si], w13u_slabs[si]):
                        pt = pst.tile([P, P], bf, tag="pt")
                        nc.tensor.transpose(pt[:], src[:, 0, 0:P], ident)
                    seen_slab = si
                pg = ps1.tile([P, T], f32, tag="pg")
                pu = ps1.tile([P, T], f32, tag="pu")
                for k in range(KH):
                    nc.tensor.matmul(
                        pg[:], w13g_slabs[si][:, k, co:co + P], xT[:, k, :],
                        start=(k == 0), stop=(k == KH - 1),
                    )
                for k in range(KH):
                    nc.tensor.matmul(
                        pu[:], w13u_slabs[si][:, k, co:co + P], xT[:, k, :],
                        start=(k == 0), stop=(k == KH - 1),
                    )
                sg = sgp.tile([P, T], bf, tag="sg")
                nc.scalar.activation(sg[:], pg[:], fn)
                # A DVE instruction may carry one sync wait: this tiny copy
                # takes the ACT wait so the gating mul below only needs PE.
                touch = tch.tile([P, 1], bf, tag="touch")
                nc.vector.tensor_copy(touch[:], sg[:, 0:1])
                gt = gtp.tile([P, T], bf, tag="gt")
                nc.vector.scalar_tensor_tensor(
                    gt[:], pu[:], 1.0, sg[:],
                    mybir.AluOpType.mult, mybir.AluOpType.mult,
                )
                gts.append(gt)

            # Observer for w2s before mm2 reads it.
            pt = pst.tile([P, P], bf, tag="pt")
            nc.tensor.transpose(pt[:], w2s[:, 0, 0:P], ident)

            # ---- mm2: down[mc*P:, :] = gatedT.T @ w2 ----
            obuf = otp.tile([P, T // P, H], bf, tag="obuf")
            for mc in range(T // P):  # 4
                for nh in range(H // FD):  # 2
                    pd = ps2.tile([P, FD], f32, tag="pd")
                    for kc in range(NI):
                        nc.tensor.matmul(
                            pd[:],
                            gts[kc][:, mc * P:(mc + 1) * P],
                            w2s[:, kc, nh * FD:(nh + 1) * FD],
                            start=(kc == 0), stop=(kc == NI - 1),
                        )
                    nc.vector.tensor_copy(
                        obuf[:, mc, nh * FD:(nh + 1) * FD], pd[:]
                    )
                if mc == 1:
                    nc.sync.dma_start(
                        out.rearrange("(a p) h -> p a h", p=P)[:, 0:2, :],
                        obuf[:, 0:2, :],
                    )
            nc.sync.dma_start(
                out.rearrange("(a p) h -> p a h", p=P)[:, 2:4, :],
                obuf[:, 2:4, :],
            )

    return nc


def _get_nc():
    if "nc" not in _NC_CACHE:
        _NC_CACHE["nc"] = _build_nc()
    return _NC_CACHE["nc"]


def kernel(bs, hiddens, w13_weight, w2_weight, batch_sizes, **_ignored):
    from concourse.bass_utils import run_bass_kernel_spmd

    hiddens = np.asarray(hiddens)
    w13_weight = np.asarray(w13_weight)
    w2_weight = np.asarray(w2_weight)
    batch_sizes = np.asarray(batch_sizes).astype(np.int64)

    in_dtype = hiddens.dtype
    x = np.ascontiguousarray(hiddens.astype(BF16))
    w13 = np.ascontiguousarray(w13_weight.astype(BF16))
    w2 = np.ascontiguousarray(w2_weight.astype(BF16))

    assert batch_sizes.shape == (E,) and int(batch_sizes.sum()) == N, (
        "kernel compiled for 8 experts x 4096 tokens"
    )

    offsets = np.concatenate([[0], np.cumsum(batch_sizes)])
    uniform = bool((batch_sizes == T).all())

    ident_rows = np.zeros((P, H), dtype=BF16)
    ident_rows[:P, :P] = np.eye(P, dtype=np.float32).astype(BF16)

    in_maps = []
    for e in range(E):
        xe = np.zeros((XA, H), dtype=BF16)
        if uniform:
            xe[:T] = x[e * T:(e + 1) * T]
        else:
            blk = x[offsets[e]:offsets[e + 1]]
            assert blk.shape[0] <= T, "per-expert batch exceeds compiled T"
            xe[: blk.shape[0]] = blk
        xe[T:] = ident_rows
        in_maps.append({"x": xe, "w13": w13[e], "w2": w2[e]})

    nc = _get_nc()
    results = run_bass_kernel_spmd(nc, in_maps, list(range(E))).results

    out_full = np.empty((N, H), dtype=BF16)
    for e in range(E):
        oe = np.asarray(results[e]["out"])
        if uniform:
            out_full[e * T:(e + 1) * T] = oe
        else:
            nb = int(batch_sizes[e])
            out_full[offsets[e]:offsets[e + 1]] = oe[:nb]

    return out_full.astype(in_dtype)
